# revision 48
# baseline (speedup 1.0000x reference)
"""Trainium2 Bass kernel for causal multi-head attention with RoPE.

Problem: B=2, S=2048, D=1024, H=16 heads, L=64 head dim, causal, interleaved
RoPE, fp32 reference.

Sharding (8 cores): data-parallel over batch (2 groups of 4 cores) x tensor
parallel over heads (4 heads per core).

v2 structure (vs v1's phase-serial + ReduceScatter design):
  - Block-pipelined front end: for each 512-row block st, project Q/K for the
    block, apply RoPE, merge RoPE halves, project V tiles, then immediately
    emit attention for qb=st.  Per-engine in-order queues then overlap the
    remaining projections (PE) with attention exp (ACT).
  - Attention: flash-style causal with transposed scores [k, q]; softmax
    denominators ride the PV matmul via a ones column appended to V (output
    row 64); two head-waves so exp overlaps PE work.  Causal masks run on
    GpSimd so the DVE queue never waits on ACT progress.
  - Output projection via AllToAll of the *normalized attended* heads
    (256 KB/qb) instead of ReduceScatter of partial outputs (1 MB/qb): each
    core receives all 16 heads' attended rows for its own 128-row slice and
    contracts the full 1024 attention columns against the full Wo.  The
    c=128 N=512 out-proj matmuls also keep the PE HAM clock-gate warm during
    the attention phase.  Finalize is split in two stages pipelined two
    blocks deep so the gpsimd queue never head-of-line blocks on collective
    completion.
  - Softmax normalization broadcast via gpsimd.partition_broadcast
    (SBUF-only; no DRAM round-trip).
Host glue: shard/permute/cast inputs, concatenate output shards, add bo.

Matmuls run in bf16 (fp32 PSUM accumulation).  The RoPE pair dimension is
host-permuted into separated halves (x0 cols then x1 cols) so on-chip RoPE is
6 dense tensor ops per tensor; the permutation is applied identically to Q and
K so dot products (scores) are unchanged.
"""

import sys

import numpy as np

for _p in ("/opt/trn_rl_repo",):
    if _p not in sys.path:
        sys.path.insert(0, _p)

import ml_dtypes

import concourse.bass as bass  # noqa: F401  (registers types)
import concourse.mybir as mybir
import concourse.tile as tile
from concourse import bacc
from concourse import bass_utils

BF16 = mybir.dt.bfloat16
F32 = mybir.dt.float32
NPBF16 = ml_dtypes.bfloat16
AF = mybir.ActivationFunctionType
ALU = mybir.AluOpType

B, S, D = 2, 2048, 1024
H, L = 16, 64
HPC = 4  # heads per core
N_CORES = 8
QB = 512  # query block
NQB = S // QB  # 4
NKT = S // 128  # 16 key tiles
ROPE_BASE = 10000.0
A2A_GROUPS = [[0, 1, 2, 3, 4, 5, 6, 7]]


def build_program():
    nc = bacc.Bacc(
        "TRN2", target_bir_lowering=False, debug=False, num_devices=N_CORES
    )

    # ---- I/O ----
    xt_d = nc.dram_tensor("xt", [D, S], BF16, kind="ExternalInput")
    wq0_d = nc.dram_tensor("wq0", [D, 128], BF16, kind="ExternalInput")
    wq1_d = nc.dram_tensor("wq1", [D, 128], BF16, kind="ExternalInput")
    wk0_d = nc.dram_tensor("wk0", [D, 128], BF16, kind="ExternalInput")
    wk1_d = nc.dram_tensor("wk1", [D, 128], BF16, kind="ExternalInput")
    wv_d = nc.dram_tensor("wv", [D, 256], BF16, kind="ExternalInput")
    wo_d = nc.dram_tensor("wo", [D, D], BF16, kind="ExternalInput")  # FULL Wo
    bq0_d = nc.dram_tensor("bq0", [128, 1], F32, kind="ExternalInput")
    bq1_d = nc.dram_tensor("bq1", [128, 1], F32, kind="ExternalInput")
    bk0_d = nc.dram_tensor("bk0", [128, 1], F32, kind="ExternalInput")
    bk1_d = nc.dram_tensor("bk1", [128, 1], F32, kind="ExternalInput")
    bvr_d = nc.dram_tensor("bvr", [1, 256], BF16, kind="ExternalInput")
    cos_d = nc.dram_tensor("cos4", [128, S], BF16, kind="ExternalInput")
    sin_d = nc.dram_tensor("sin4", [128, S], BF16, kind="ExternalInput")
    tri_d = nc.dram_tensor("tri2", [128, 256], BF16, kind="ExternalInput")
    out_d = nc.dram_tensor("out", [4 * 128, D], BF16, kind="ExternalOutput")

    # AllToAll buffers over all 8 cores: [dest/src rank, 256 attcols
    # (h*64+l), 64 rows].  Each core owns 64 output rows of EACH batch per
    # query block, so every exchanged chunk is useful (mesh needs >4 cores).
    a2a_in_d = [
        nc.dram_tensor(f"a2ai{qb}", [8, 256, 64], BF16, kind="Internal")
        for qb in range(NQB)
    ]
    a2a_out_d = [
        nc.dram_tensor(f"a2ao{qb}", [8, 256, 64], BF16, kind="Internal")
        for qb in range(NQB)
    ]

    with tile.TileContext(nc) as tc:
        with (
            tc.tile_pool(name="const", bufs=1) as cpool,
            tc.tile_pool(name="xp", bufs=1) as xpool,
            tc.tile_pool(name="qk", bufs=1) as qkpool,
            tc.tile_pool(name="rtmp", bufs=1) as rtmp,
            tc.tile_pool(name="ptp", bufs=3) as ptpool,
            tc.tile_pool(name="att", bufs=1) as attpool,
            tc.tile_pool(name="bc", bufs=2) as bcpool,
            tc.tile_pool(name="afp", bufs=2) as afpool,
            tc.tile_pool(name="osb", bufs=2) as opool,
            # PSUM: one shared double-buffered [128,2,512] pool (4 banks)
            # serves scores AND proj/outproj (which use a slice of it), + pv
            # (4 banks) = 8 banks exactly.  Double-buffering the score tiles
            # removes the exp(w) -> scores(w+1) serial edge: consecutive
            # waves land in alternating buffers, so the PE computes the next
            # wave's scores while ACT still reads the previous one.
            tc.tile_pool(name="pscp", bufs=2, space="PSUM") as pscp,
            tc.tile_pool(name="ppvp", bufs=1, space="PSUM") as ppvp,
        ):
            # bulky constants that aren't needed until RoPE / masks / outproj
            # go on the gpsimd (SWDGE) queue so the SP queue reaches the
            # first projection matmul's inputs quickly
            def load_g(dram, shape, dt, tag):
                t = cpool.tile(shape, dt, tag=tag)
                nc.gpsimd.dma_start(t[:], dram.ap())
                return t

            cos_sb = load_g(cos_d, [128, S], BF16, "cos4")
            sin_sb = load_g(sin_d, [128, S], BF16, "sin4")
            tri2_sb = load_g(tri_d, [128, 256], BF16, "tri2")
            # wo is loaded later (wo_unit) so it doesn't compete for HBM
            # bandwidth with the startup x/weight loads
            wo_sb = cpool.tile([128, 8, D], BF16)
            tri2_v = tri2_sb[:].rearrange("p (w c) -> p w c", w=2)

            ones_row = cpool.tile([1, 128], BF16, tag="ones_row")
            nc.vector.memset(ones_row[:], 1.0)

            # x^T in SBUF: block 0 chunked per dt so block-0 projections can
            # start after ~1 MB of DMA; the rest follows after the block-0
            # merges are enqueued.  The first proj matmul needs x chunk 0 +
            # wq0, so those DMAs go absolutely first on the SP queue.
            xt_sb = xpool.tile([128, 8, S], BF16)
            xt_r = xt_d.ap().rearrange("(o p) s -> p o s", p=128)
            nc.sync.dma_start(xt_sb[:, 0, 0:512], xt_r[:, 0, 0:512])

            def load_w(dram, cols):
                t = cpool.tile([128, 8, cols], BF16, tag=f"w_{dram.name}")
                nc.sync.dma_start(t[:], dram.ap().rearrange("(o p) m -> p o m", p=128))
                return t

            wq0_sb = load_w(wq0_d, 128)
            for dt_ in range(1, 8):
                nc.sync.dma_start(
                    xt_sb[:, dt_, 0:512], xt_r[:, dt_, 0:512]
                )
            wq1_sb = load_w(wq1_d, 128)
            wk0_sb = load_w(wk0_d, 128)
            wk1_sb = load_w(wk1_d, 128)

            def load_c(dram, shape, dt, tag):
                t = cpool.tile(shape, dt, tag=tag)
                nc.sync.dma_start(t[:], dram.ap())
                return t

            bq0_sb = load_c(bq0_d, [128, 1], F32, "bq0")
            bq1_sb = load_c(bq1_d, [128, 1], F32, "bq1")
            bk0_sb = load_c(bk0_d, [128, 1], F32, "bk0")
            bk1_sb = load_c(bk1_d, [128, 1], F32, "bk1")
            bvr_sb = load_c(bvr_d, [1, 256], BF16, "bvr")

            def load_x_rest():
                for dt_ in range(8):
                    nc.sync.dma_start(
                        xt_sb[:, dt_, 512:S], xt_r[:, dt_, 512:S]
                    )

            wv_sb = load_w(wv_d, 256)

            # ---- persistent SBUF tensors ----
            q0_sb = qkpool.tile([128, S], BF16, tag="q0")
            q1_sb = qkpool.tile([128, S], BF16, tag="q1")
            k0_sb = qkpool.tile([128, S], BF16, tag="k0")
            k1_sb = qkpool.tile([128, S], BF16, tag="k1")
            qm = [
                qkpool.tile([128, S], BF16, tag=f"qm{w}", name=f"qm{w}")
                for w in range(2)
            ]
            km = [
                qkpool.tile([128, S], BF16, tag=f"km{w}", name=f"km{w}")
                for w in range(2)
            ]
            # V padded to 128 cols/head: [64 attended | ones | zeros].  The
            # ones column makes the PV matmul emit the softmax denominator in
            # output row 64; the zero pad makes every PV a full-array c=128
            # 128-col matmul, which keeps the PE HAM clock-gate open through
            # the attention phase (measured: dense full-array streams run at
            # ~1.95 GHz vs 1.2 GHz throttled).
            v_sb = qkpool.tile([128, NKT, HPC * 128], BF16, tag="v")
            nc.vector.memset(
                v_sb[:].rearrange("p t (h c) -> p t h c", c=128)[:, :, :, 64:128], 0.0
            )
            nc.vector.memset(
                v_sb[:].rearrange("p t (h c) -> p t h c", c=128)[:, :, :, 64:65], 1.0
            )

            # attended^T + denominators: row 64 of each head's PV output
            attg_sb = attpool.tile([65, HPC, S], BF16, tag="attg")

            DSTS = (
                (q0_sb, wq0_sb, bq0_sb),
                (q1_sb, wq1_sb, bq1_sb),
                (k0_sb, wk0_sb, bk0_sb),
                (k1_sb, wk1_sb, bk1_sb),
            )

            # ---- work units (closures): emitted either directly or as
            # fillers between attention kt-iterations so the in-order PE
            # queue always has ready c=128 work during exp waits ----

            def qk_unit(st, idx):
                def emit():
                    dst, w_sb, b_sb = DSTS[idx]
                    sl = slice(st * 512, (st + 1) * 512)
                    psq = pscp.tile(
                        [128, 2, 512], F32, tag="psc", name=f"pp_{st}_{idx}"
                    )
                    ps = psq[:, 0, :]
                    for dt_ in range(8):
                        nc.tensor.matmul(
                            ps[:],
                            w_sb[:, dt_, :],
                            xt_sb[:, dt_, sl],
                            start=(dt_ == 0),
                            stop=(dt_ == 7),
                        )
                    nc.vector.tensor_scalar(
                        dst[:, sl], ps[:], b_sb[:, 0:1], None, ALU.add
                    )
                return emit

            def rope_merge_unit(st, dma=None):
                def emit():
                    dma_start = dma if dma is not None else nc.sync.dma_start
                    sl = slice(st * 512, (st + 1) * 512)
                    # RoPE on DVE for this block
                    for x0, x1 in ((q0_sb, q1_sb), (k0_sb, k1_sb)):
                        m1 = rtmp.tile([128, S], BF16, tag="m1", name=f"m1_{st}")
                        m2 = rtmp.tile([128, S], BF16, tag="m2", name=f"m2_{st}")
                        m3 = rtmp.tile([128, S], BF16, tag="m3", name=f"m3_{st}")
                        m4 = rtmp.tile([128, S], BF16, tag="m4", name=f"m4_{st}")
                        nc.vector.tensor_tensor(m1[:, sl], x0[:, sl], cos_sb[:, sl], ALU.mult)
                        nc.vector.tensor_tensor(m2[:, sl], x1[:, sl], sin_sb[:, sl], ALU.mult)
                        nc.vector.tensor_tensor(m3[:, sl], x0[:, sl], sin_sb[:, sl], ALU.mult)
                        nc.vector.tensor_tensor(m4[:, sl], x1[:, sl], cos_sb[:, sl], ALU.mult)
                        nc.vector.tensor_tensor(x0[:, sl], m1[:, sl], m2[:, sl], ALU.subtract)
                        nc.vector.tensor_tensor(x1[:, sl], m3[:, sl], m4[:, sl], ALU.add)
                    # Merge the RoPE'd halves into per-head-contiguous layouts
                    # via SBUF->SBUF DMA (partition remap): head h of
                    # pair-buffer w holds rows 64h..64h+64 = [x0_h | x1_h], so
                    # each score matmul is a single c=64 contraction.
                    for w in range(2):
                        for hh in range(2):
                            h = 2 * w + hh
                            dma_start(
                                qm[w][64 * hh : 64 * hh + 32, sl],
                                q0_sb[32 * h : 32 * h + 32, sl],
                            )
                            dma_start(
                                qm[w][64 * hh + 32 : 64 * hh + 64, sl],
                                q1_sb[32 * h : 32 * h + 32, sl],
                            )
                            dma_start(
                                km[w][64 * hh : 64 * hh + 32, sl],
                                k0_sb[32 * h : 32 * h + 32, sl],
                            )
                            dma_start(
                                km[w][64 * hh + 32 : 64 * hh + 64, sl],
                                k1_sb[32 * h : 32 * h + 32, sl],
                            )
                return emit

            def v_unit(kt):
                def emit():
                    psq = pscp.tile(
                        [128, 2, 512], F32, tag="psc", name=f"pv_{kt}"
                    )
                    psv = psq[:, 0, 0:256]
                    for dt_ in range(8):
                        nc.tensor.matmul(
                            psv,
                            xt_sb[:, dt_, kt * 128 : (kt + 1) * 128],
                            wv_sb[:, dt_, :],
                            start=(dt_ == 0),
                            stop=False,
                        )
                    nc.tensor.matmul(
                        psv, ones_row[0:1, :], bvr_sb[0:1, :], start=False, stop=True
                    )
                    nc.vector.tensor_copy(
                        v_sb[:, kt, :].rearrange("p (h c) -> p h c", c=128)[:, :, 0:64],
                        psv.rearrange("p (h c) -> p h c", c=64),
                    )
                return emit

            def outproj_fillers(qb):
                """Two PE units: AllToAll gather + out-projection halves."""
                state = {}

                def f0():
                    af = afpool.tile([128, 8, 128], BF16, tag="af", name=f"af_{qb}")
                    for z in range(2):
                        nc.gpsimd.dma_start(
                            af[:, :, z * 64 : (z + 1) * 64],
                            a2a_out_d[qb][4 * z : 4 * z + 4]
                            .rearrange("s (r p) q -> p (s r) q", p=128),
                        )
                    osb_t = opool.tile([128, D], BF16, tag="osb", name=f"osb_{qb}")
                    state["af"], state["osb"] = af, osb_t
                    po = pscp.tile(
                        [128, 2, 512], F32, tag="psc", name=f"po_{qb}_0"
                    )[:, 0, :]
                    for k in range(8):
                        nc.tensor.matmul(
                            po[:],
                            af[:, k, :],
                            wo_sb[:, k, 0:512],
                            start=(k == 0),
                            stop=(k == 7),
                        )
                    nc.vector.tensor_copy(osb_t[:, 0:512], po[:])

                def f1():
                    af, osb_t = state["af"], state["osb"]
                    po = pscp.tile(
                        [128, 2, 512], F32, tag="psc", name=f"po_{qb}_1"
                    )[:, 0, :]
                    for k in range(8):
                        nc.tensor.matmul(
                            po[:],
                            af[:, k, :],
                            wo_sb[:, k, 512:D],
                            start=(k == 0),
                            stop=(k == 7),
                        )
                    nc.vector.tensor_copy(osb_t[:, 512:D], po[:])
                    nc.gpsimd.dma_start(out_d[qb * 128 : (qb + 1) * 128, :], osb_t[:])

                return [f0, f1]

            def attention_block(qb, fillers=(), chunk_cb=None, pvs_ref=None):
                """Causal attention for query block qb (keys 0..(qb+1)*512).

                Pops one pending filler unit after each kt iteration (evenly
                spread) so the PE queue interleaves ready full-array matmuls
                with the exp-gated attention matmuls.  chunk_cb(c) fires once
                128-query chunk c's last PV is emitted (diagonal pipelining).
                """
                fillers = list(fillers)
                popped = 0
                pvs = [
                    ppvp.tile([128, 512], F32, tag=f"pv{h}", name=f"pv{h}_{qb}")
                    for h in range(HPC)
                ]
                if pvs_ref is not None:
                    pvs_ref[0] = pvs
                nkt = 4 * qb + 4

                # PV runs one wave behind: the PE queue goes
                # [sc(w)][PV(w-1)][sc(w+1)][PV(w)]... so exp(w) (ACT) overlaps
                # PV(w-1) instead of serializing with it.
                pend_pv = []  # (kt, w, pt, qlo)

                def flush_pv():
                    while pend_pv:
                        pkt, pw, ppt, pqlo = pend_pv.pop(0)
                        for hh in range(2):
                            h = 2 * pw + hh
                            nc.tensor.matmul(
                                pvs[h][:, pqlo:512],
                                v_sb[:, pkt, 128 * h : 128 * h + 128],
                                ppt[:, hh, pqlo:512],
                                start=(pkt == 0),
                                stop=(pkt == nkt - 1),
                            )

                for kt in range(nkt):
                    j = kt - 4 * qb  # >= 0 on diagonal tiles
                    qlo = max(0, j * 128)
                    g0 = qb * 512 + qlo
                    g1 = (qb + 1) * 512
                    for w in range(2):
                        psc = pscp.tile(
                            [128, 2, 512], F32, tag="psc", name=f"psc{w}_{qb}_{kt}"
                        )
                        for hh in range(2):
                            nc.tensor.matmul(
                                psc[:, hh, qlo:512],
                                km[w][
                                    64 * hh : 64 * hh + 64,
                                    kt * 128 : (kt + 1) * 128,
                                ],
                                qm[w][64 * hh : 64 * hh + 64, g0:g1],
                                start=True,
                                stop=True,
                                tile_position=(64 * hh, 0),
                            )
                        pt = ptpool.tile(
                            [128, 2, 512], BF16, tag="pt", name=f"pt{w}_{qb}_{kt}"
                        )
                        nc.scalar.activation(
                            pt[:, :, qlo:512], psc[:, :, qlo:512], AF.Exp, scale=0.125
                        )
                        if j >= 0:
                            # causal mask on GpSimd: keeps the DVE queue free
                            # of ACT-dependent work (PV-drain ordering)
                            nc.gpsimd.tensor_tensor(
                                pt[:, :, qlo : qlo + 128],
                                pt[:, :, qlo : qlo + 128],
                                tri2_v,
                                ALU.mult,
                            )
                        flush_pv()
                        if chunk_cb is not None and j >= 1 and w == 0:
                            # chunk j-1's last PV was just flushed (w0)
                            chunk_cb(j - 1)
                        pend_pv.append((kt, w, pt, qlo))
                    while fillers and popped < (kt + 1) * len(fillers) // nkt:
                        fillers[popped]()
                        popped += 1
                flush_pv()
                if chunk_cb is not None:
                    chunk_cb(3)
                while popped < len(fillers):
                    fillers[popped]()
                    popped += 1
                return pvs

            def make_chunk_cb(qb, pvs_ref):
                """Per-128-query-chunk finalize: drain + recip + normalize +
                stage, pipelined with the block's remaining diagonal tiles.
                The AllToAll launches right after chunk 3."""

                def cb(c):
                    pvs = pvs_ref[0]
                    q0 = qb * 512 + c * 128
                    sl = slice(q0, q0 + 128)
                    for h in range(HPC):
                        nc.vector.tensor_copy(
                            attg_sb[:, h, sl], pvs[h][0:65, c * 128 : (c + 1) * 128]
                        )
                    sums_t = bcpool.tile(
                        [HPC, 128], BF16, tag="sums", name=f"sums_{qb}_{c}"
                    )
                    nc.sync.dma_start(sums_t[:], attg_sb[64:65, :, sl])
                    rb = bcpool.tile([HPC, 128], BF16, tag="rb", name=f"rb_{qb}_{c}")
                    with nc.allow_low_precision(reason="bf16 softmax denom recip"):
                        nc.vector.reciprocal(rb[:], sums_t[:])
                    rrow = bcpool.tile(
                        [1, HPC, 128], BF16, tag="rrow", name=f"rr_{qb}_{c}"
                    )
                    nc.sync.dma_start(rrow[:], rb[:])
                    bcx = bcpool.tile(
                        [64, HPC, 128], BF16, tag="bcx", name=f"bcx_{qb}_{c}"
                    )
                    nc.gpsimd.partition_broadcast(bcx[:], rrow[:])
                    for h in range(HPC):
                        slh = attg_sb[0:64, h, sl]
                        nc.vector.tensor_tensor(slh, slh, bcx[:, h, :], ALU.mult)
                    # stage the two AllToAll chunks covered by these queries
                    for p in (2 * c, 2 * c + 1):
                        nc.sync.dma_start(
                            a2a_in_d[qb][p].rearrange("(h l) q -> l h q", l=64),
                            attg_sb[
                                0:64, :, qb * 512 + p * 64 : qb * 512 + (p + 1) * 64
                            ],
                        )
                    if c == 3:
                        nc.gpsimd.collective_compute(
                            "AllToAll",
                            ALU.bypass,
                            replica_groups=A2A_GROUPS,
                            ins=[a2a_in_d[qb][:]],
                            outs=[a2a_out_d[qb][:]],
                        )

                return cb

            def wo_unit():
                nc.gpsimd.dma_start(
                    wo_sb[:], wo_d.ap().rearrange("(o p) m -> p o m", p=128)
                )

            # ---- emission ----
            # Prologue: block-0 Q/K projections + RoPE/merge + first V tile;
            # the ACT exp-table preload runs while the PE is dense.
            for idx in range(4):
                qk_unit(0, idx)()
            warm_act = cpool.tile([128, 1], F32, tag="warm_act")
            nc.scalar.activation(warm_act[:], bq0_sb[:], AF.Exp)
            rope_merge_unit(0)()
            load_x_rest()
            v_unit(0)()

            # attention(st) carries the remaining V tiles, the next block's
            # projection units, and trailing out-projections as fillers; the
            # per-chunk callback drains/normalizes/stages as the causal
            # diagonal completes and fires the AllToAll at chunk 3.
            ref = [None]
            attention_block(
                0,
                [v_unit(kt) for kt in range(1, 4)]
                + [qk_unit(1, i) for i in range(4)]
                + [rope_merge_unit(1)]
                + [v_unit(kt) for kt in range(4, 8)],
                chunk_cb=make_chunk_cb(0, ref),
                pvs_ref=ref,
            )
            ref = [None]
            attention_block(
                1,
                [qk_unit(2, i) for i in range(4)]
                + [rope_merge_unit(2), wo_unit]
                + [v_unit(kt) for kt in range(8, 12)],
                chunk_cb=make_chunk_cb(1, ref),
                pvs_ref=ref,
            )
            ref = [None]
            attention_block(
                2,
                [qk_unit(3, i) for i in range(4)]
                + [rope_merge_unit(3)]
                + [v_unit(kt) for kt in range(12, 16)]
                + outproj_fillers(0),
                chunk_cb=make_chunk_cb(2, ref),
                pvs_ref=ref,
            )
            ref = [None]
            attention_block(
                3,
                outproj_fillers(1) + outproj_fillers(2),
                chunk_cb=make_chunk_cb(3, ref),
                pvs_ref=ref,
            )
            for f in outproj_fillers(3):
                f()

    nc.compile()
    return nc


def make_in_maps(x, Wq, bq, Wk, bk, Wv, bv, Wo):
    inv = 1.0 / (ROPE_BASE ** (2.0 * np.arange(32, dtype=np.float64) / L))
    ang = np.arange(S, dtype=np.float64)[:, None] * inv[None, :]  # [S, 32]
    cos4 = np.tile(np.cos(ang).T, (HPC, 1)).astype(NPBF16)  # [128, S]
    sin4 = np.tile(np.sin(ang).T, (HPC, 1)).astype(NPBF16)
    tri = (np.arange(128)[None, :] >= np.arange(128)[:, None]).astype(NPBF16)
    tri2 = np.repeat(tri[:, None, :], 2, axis=1).reshape(128, 256)
    wo_full = np.ascontiguousarray(Wo).astype(NPBF16)

    in_maps = []
    for c in range(N_CORES):
        b, g = divmod(c, HPC)
        even = np.concatenate(
            [64 * h + 2 * np.arange(32) for h in range(4 * g, 4 * g + 4)]
        )
        odd = even + 1
        vcols = np.arange(256 * g, 256 * (g + 1))
        in_maps.append(
            {
                "xt": np.ascontiguousarray(x[b].T).astype(NPBF16),
                "wq0": np.ascontiguousarray(Wq[:, even]).astype(NPBF16),
                "wq1": np.ascontiguousarray(Wq[:, odd]).astype(NPBF16),
                "wk0": np.ascontiguousarray(Wk[:, even]).astype(NPBF16),
                "wk1": np.ascontiguousarray(Wk[:, odd]).astype(NPBF16),
                "wv": np.ascontiguousarray(Wv[:, vcols]).astype(NPBF16),
                "wo": wo_full,
                "bq0": bq[even].reshape(128, 1).astype(np.float32),
                "bq1": bq[odd].reshape(128, 1).astype(np.float32),
                "bk0": bk[even].reshape(128, 1).astype(np.float32),
                "bk1": bk[odd].reshape(128, 1).astype(np.float32),
                "bvr": bv[vcols].reshape(1, 256).astype(NPBF16),
                "cos4": cos4,
                "sin4": sin4,
                "tri2": tri2,
            }
        )
    return in_maps


def assemble_output(results, bo):
    out = np.empty((B, S, D), np.float32)
    for c in range(N_CORES):
        # core c owns rows qb*512 + c*64 .. +64 of BOTH batches
        sh = np.asarray(results[c]["out"]).astype(np.float32).reshape(NQB, 2, 64, D)
        for qb in range(NQB):
            r0 = qb * 512 + c * 64
            out[0, r0 : r0 + 64, :] = sh[qb, 0]
            out[1, r0 : r0 + 64, :] = sh[qb, 1]
    # bo is added once, after the reduction (matches `attended @ Wo + bo`).
    out += bo[None, None, :].astype(np.float32)
    return out


_CACHE = {}


def kernel(x, Wq, bq, Wk, bk, Wv, bv, Wo, bo, **run_kwargs):
    if "nc" not in _CACHE:
        _CACHE["nc"] = build_program()
    nc = _CACHE["nc"]
    in_maps = make_in_maps(
        np.asarray(x), np.asarray(Wq), np.asarray(bq), np.asarray(Wk),
        np.asarray(bk), np.asarray(Wv), np.asarray(bv), np.asarray(Wo),
    )
    res = bass_utils.run_bass_kernel_spmd(
        nc, in_maps, core_ids=list(range(N_CORES)), **run_kwargs
    )
    out = assemble_output(res.results, np.asarray(bo))
    kernel.last_results = res
    return out


# revision 49
# speedup vs baseline: 1.1283x; 1.1283x over previous
"""Trainium2 Bass kernel for causal multi-head attention with RoPE.

Problem: B=2, S=2048, D=1024, H=16 heads, L=64 head dim, causal, interleaved
RoPE, fp32 reference.

Sharding (8 cores): data-parallel over batch (2 groups of 4 cores) x tensor
parallel over heads (4 heads per core).

v2 structure (vs v1's phase-serial + ReduceScatter design):
  - Block-pipelined front end: for each 512-row block st, project Q/K for the
    block, apply RoPE, merge RoPE halves, project V tiles, then immediately
    emit attention for qb=st.  Per-engine in-order queues then overlap the
    remaining projections (PE) with attention exp (ACT).
  - Attention: flash-style causal with transposed scores [k, q]; softmax
    denominators ride the PV matmul via a ones column appended to V (output
    row 64); two head-waves so exp overlaps PE work.  Causal masks run on
    GpSimd so the DVE queue never waits on ACT progress.
  - Output projection via AllToAll of the *normalized attended* heads
    (256 KB/qb) instead of ReduceScatter of partial outputs (1 MB/qb): each
    core receives all 16 heads' attended rows for its own 128-row slice and
    contracts the full 1024 attention columns against the full Wo.  The
    c=128 N=512 out-proj matmuls also keep the PE HAM clock-gate warm during
    the attention phase.  Finalize is split in two stages pipelined two
    blocks deep so the gpsimd queue never head-of-line blocks on collective
    completion.
  - Softmax normalization broadcast via gpsimd.partition_broadcast
    (SBUF-only; no DRAM round-trip).
Host glue: shard/permute/cast inputs, concatenate output shards, add bo.

Matmuls run in bf16 (fp32 PSUM accumulation).  The RoPE pair dimension is
host-permuted into separated halves (x0 cols then x1 cols) so on-chip RoPE is
6 dense tensor ops per tensor; the permutation is applied identically to Q and
K so dot products (scores) are unchanged.
"""

import sys

import numpy as np

for _p in ("/opt/trn_rl_repo",):
    if _p not in sys.path:
        sys.path.insert(0, _p)

import ml_dtypes

import concourse.bass as bass  # noqa: F401  (registers types)
import concourse.mybir as mybir
import concourse.tile as tile
from concourse import bacc
from concourse import bass_utils

BF16 = mybir.dt.bfloat16
F32 = mybir.dt.float32
NPBF16 = ml_dtypes.bfloat16
AF = mybir.ActivationFunctionType
ALU = mybir.AluOpType

B, S, D = 2, 2048, 1024
H, L = 16, 64
HPC = 4  # heads per core
N_CORES = 8
QB = 512  # query block
NQB = S // QB  # 4
NKT = S // 128  # 16 key tiles
ROPE_BASE = 10000.0
A2A_GROUPS = [[0, 1, 2, 3, 4, 5, 6, 7]]


def build_program():
    nc = bacc.Bacc(
        "TRN2", target_bir_lowering=False, debug=False, num_devices=N_CORES
    )

    # ---- I/O ----
    xt_d = nc.dram_tensor("xt", [D, S], BF16, kind="ExternalInput")
    wq0_d = nc.dram_tensor("wq0", [D, 128], BF16, kind="ExternalInput")
    wq1_d = nc.dram_tensor("wq1", [D, 128], BF16, kind="ExternalInput")
    wk0_d = nc.dram_tensor("wk0", [D, 128], BF16, kind="ExternalInput")
    wk1_d = nc.dram_tensor("wk1", [D, 128], BF16, kind="ExternalInput")
    wv_d = nc.dram_tensor("wv", [D, 256], BF16, kind="ExternalInput")
    wo_d = nc.dram_tensor("wo", [D, D], BF16, kind="ExternalInput")  # FULL Wo
    bq0_d = nc.dram_tensor("bq0", [128, 1], F32, kind="ExternalInput")
    bq1_d = nc.dram_tensor("bq1", [128, 1], F32, kind="ExternalInput")
    bk0_d = nc.dram_tensor("bk0", [128, 1], F32, kind="ExternalInput")
    bk1_d = nc.dram_tensor("bk1", [128, 1], F32, kind="ExternalInput")
    bvr_d = nc.dram_tensor("bvr", [1, 256], BF16, kind="ExternalInput")
    cos_d = nc.dram_tensor("cos4", [128, S], BF16, kind="ExternalInput")
    sin_d = nc.dram_tensor("sin4", [128, S], BF16, kind="ExternalInput")
    tri_d = nc.dram_tensor("tri2", [128, 256], BF16, kind="ExternalInput")
    out_d = nc.dram_tensor("out", [4 * 128, D], BF16, kind="ExternalOutput")

    # AllToAll buffers over all 8 cores: [dest/src rank, 256 attcols
    # (h*64+l), 64 rows].  Each core owns 64 output rows of EACH batch per
    # query block, so every exchanged chunk is useful (mesh needs >4 cores).
    a2a_in_d = [
        nc.dram_tensor(f"a2ai{qb}", [8, 256, 64], BF16, kind="Internal")
        for qb in range(NQB)
    ]
    a2a_out_d = [
        nc.dram_tensor(f"a2ao{qb}", [8, 256, 64], BF16, kind="Internal")
        for qb in range(NQB)
    ]

    with tile.TileContext(nc) as tc:
        with (
            tc.tile_pool(name="const", bufs=1) as cpool,
            tc.tile_pool(name="xp", bufs=1) as xpool,
            tc.tile_pool(name="qk", bufs=1) as qkpool,
            tc.tile_pool(name="rtmp", bufs=1) as rtmp,
            tc.tile_pool(name="ptp", bufs=3) as ptpool,
            tc.tile_pool(name="att", bufs=1) as attpool,
            tc.tile_pool(name="bc", bufs=2) as bcpool,
            tc.tile_pool(name="afp", bufs=2) as afpool,
            tc.tile_pool(name="osb", bufs=2) as opool,
            # PSUM: ps512 (proj + outproj, 2 banks) + psc (2 banks)
            # + pv (4 banks) = 8 banks exactly.
            tc.tile_pool(name="ps512", bufs=2, space="PSUM") as ps512,
            tc.tile_pool(name="pscp", bufs=1, space="PSUM") as pscp,
            tc.tile_pool(name="ppvp", bufs=1, space="PSUM") as ppvp,
        ):
            # bulky constants that aren't needed until RoPE / masks / outproj
            # go on the gpsimd (SWDGE) queue so the SP queue reaches the
            # first projection matmul's inputs quickly
            def load_g(dram, shape, dt, tag):
                t = cpool.tile(shape, dt, tag=tag)
                nc.gpsimd.dma_start(t[:], dram.ap())
                return t

            cos_sb = load_g(cos_d, [128, S], BF16, "cos4")
            sin_sb = load_g(sin_d, [128, S], BF16, "sin4")
            tri2_sb = load_g(tri_d, [128, 256], BF16, "tri2")
            # wo is loaded later (wo_unit) so it doesn't compete for HBM
            # bandwidth with the startup x/weight loads
            wo_sb = cpool.tile([128, 8, D], BF16)
            tri2_v = tri2_sb[:].rearrange("p (w c) -> p w c", w=2)

            ones_row = cpool.tile([1, 128], BF16, tag="ones_row")
            nc.vector.memset(ones_row[:], 1.0)

            # x^T in SBUF: block 0 chunked per dt so block-0 projections can
            # start after ~1 MB of DMA; the rest follows after the block-0
            # merges are enqueued.  The first proj matmul needs x chunk 0 +
            # wq0, so those DMAs go absolutely first on the SP queue.
            xt_sb = xpool.tile([128, 8, S], BF16)
            xt_r = xt_d.ap().rearrange("(o p) s -> p o s", p=128)
            nc.sync.dma_start(xt_sb[:, 0, 0:512], xt_r[:, 0, 0:512])

            def load_w(dram, cols):
                t = cpool.tile([128, 8, cols], BF16, tag=f"w_{dram.name}")
                nc.sync.dma_start(t[:], dram.ap().rearrange("(o p) m -> p o m", p=128))
                return t

            wq0_sb = load_w(wq0_d, 128)
            for dt_ in range(1, 8):
                nc.sync.dma_start(
                    xt_sb[:, dt_, 0:512], xt_r[:, dt_, 0:512]
                )
            wq1_sb = load_w(wq1_d, 128)
            wk0_sb = load_w(wk0_d, 128)
            wk1_sb = load_w(wk1_d, 128)

            def load_c(dram, shape, dt, tag):
                t = cpool.tile(shape, dt, tag=tag)
                nc.sync.dma_start(t[:], dram.ap())
                return t

            bq0_sb = load_c(bq0_d, [128, 1], F32, "bq0")
            bq1_sb = load_c(bq1_d, [128, 1], F32, "bq1")
            bk0_sb = load_c(bk0_d, [128, 1], F32, "bk0")
            bk1_sb = load_c(bk1_d, [128, 1], F32, "bk1")
            bvr_sb = load_c(bvr_d, [1, 256], BF16, "bvr")

            def load_x_rest():
                for dt_ in range(8):
                    nc.sync.dma_start(
                        xt_sb[:, dt_, 512:S], xt_r[:, dt_, 512:S]
                    )

            wv_sb = load_w(wv_d, 256)

            # ---- persistent SBUF tensors ----
            q0_sb = qkpool.tile([128, S], BF16, tag="q0")
            q1_sb = qkpool.tile([128, S], BF16, tag="q1")
            k0_sb = qkpool.tile([128, S], BF16, tag="k0")
            k1_sb = qkpool.tile([128, S], BF16, tag="k1")
            qm = [
                qkpool.tile([128, S], BF16, tag=f"qm{w}", name=f"qm{w}")
                for w in range(2)
            ]
            km = [
                qkpool.tile([128, S], BF16, tag=f"km{w}", name=f"km{w}")
                for w in range(2)
            ]
            # V padded to 128 cols/head: [64 attended | ones | zeros].  The
            # ones column makes the PV matmul emit the softmax denominator in
            # output row 64; the zero pad makes every PV a full-array c=128
            # 128-col matmul, which keeps the PE HAM clock-gate open through
            # the attention phase (measured: dense full-array streams run at
            # ~1.95 GHz vs 1.2 GHz throttled).
            v_sb = qkpool.tile([128, NKT, HPC * 128], BF16, tag="v")
            nc.vector.memset(
                v_sb[:].rearrange("p t (h c) -> p t h c", c=128)[:, :, :, 64:128], 0.0
            )
            nc.vector.memset(
                v_sb[:].rearrange("p t (h c) -> p t h c", c=128)[:, :, :, 64:65], 1.0
            )

            # attended^T + denominators: row 64 of each head's PV output
            attg_sb = attpool.tile([65, HPC, S], BF16, tag="attg")

            DSTS = (
                (q0_sb, wq0_sb, bq0_sb),
                (q1_sb, wq1_sb, bq1_sb),
                (k0_sb, wk0_sb, bk0_sb),
                (k1_sb, wk1_sb, bk1_sb),
            )

            # ---- work units (closures): emitted either directly or as
            # fillers between attention kt-iterations so the in-order PE
            # queue always has ready c=128 work during exp waits ----

            def qk_unit(st, idx):
                def emit():
                    dst, w_sb, b_sb = DSTS[idx]
                    sl = slice(st * 512, (st + 1) * 512)
                    ps = ps512.tile([128, 512], F32, tag="pp", name=f"pp_{st}_{idx}")
                    for dt_ in range(8):
                        nc.tensor.matmul(
                            ps[:],
                            w_sb[:, dt_, :],
                            xt_sb[:, dt_, sl],
                            start=(dt_ == 0),
                            stop=(dt_ == 7),
                        )
                    nc.vector.tensor_scalar(
                        dst[:, sl], ps[:], b_sb[:, 0:1], None, ALU.add
                    )
                return emit

            def rope_merge_unit(st, dma=None):
                def emit():
                    dma_start = dma if dma is not None else nc.sync.dma_start
                    sl = slice(st * 512, (st + 1) * 512)
                    # RoPE on DVE for this block
                    for x0, x1 in ((q0_sb, q1_sb), (k0_sb, k1_sb)):
                        m1 = rtmp.tile([128, S], BF16, tag="m1", name=f"m1_{st}")
                        m2 = rtmp.tile([128, S], BF16, tag="m2", name=f"m2_{st}")
                        m3 = rtmp.tile([128, S], BF16, tag="m3", name=f"m3_{st}")
                        m4 = rtmp.tile([128, S], BF16, tag="m4", name=f"m4_{st}")
                        nc.vector.tensor_tensor(m1[:, sl], x0[:, sl], cos_sb[:, sl], ALU.mult)
                        nc.vector.tensor_tensor(m2[:, sl], x1[:, sl], sin_sb[:, sl], ALU.mult)
                        nc.vector.tensor_tensor(m3[:, sl], x0[:, sl], sin_sb[:, sl], ALU.mult)
                        nc.vector.tensor_tensor(m4[:, sl], x1[:, sl], cos_sb[:, sl], ALU.mult)
                        nc.vector.tensor_tensor(x0[:, sl], m1[:, sl], m2[:, sl], ALU.subtract)
                        nc.vector.tensor_tensor(x1[:, sl], m3[:, sl], m4[:, sl], ALU.add)
                    # Merge the RoPE'd halves into per-head-contiguous layouts
                    # via SBUF->SBUF DMA (partition remap): head h of
                    # pair-buffer w holds rows 64h..64h+64 = [x0_h | x1_h], so
                    # each score matmul is a single c=64 contraction.
                    for w in range(2):
                        for hh in range(2):
                            h = 2 * w + hh
                            dma_start(
                                qm[w][64 * hh : 64 * hh + 32, sl],
                                q0_sb[32 * h : 32 * h + 32, sl],
                            )
                            dma_start(
                                qm[w][64 * hh + 32 : 64 * hh + 64, sl],
                                q1_sb[32 * h : 32 * h + 32, sl],
                            )
                            dma_start(
                                km[w][64 * hh : 64 * hh + 32, sl],
                                k0_sb[32 * h : 32 * h + 32, sl],
                            )
                            dma_start(
                                km[w][64 * hh + 32 : 64 * hh + 64, sl],
                                k1_sb[32 * h : 32 * h + 32, sl],
                            )
                return emit

            def v_unit(kt):
                def emit():
                    ps = ps512.tile([128, 512], F32, tag="pp", name=f"pv_{kt}")
                    psv = ps[:, 0:256]
                    for dt_ in range(8):
                        nc.tensor.matmul(
                            psv,
                            xt_sb[:, dt_, kt * 128 : (kt + 1) * 128],
                            wv_sb[:, dt_, :],
                            start=(dt_ == 0),
                            stop=False,
                        )
                    nc.tensor.matmul(
                        psv, ones_row[0:1, :], bvr_sb[0:1, :], start=False, stop=True
                    )
                    nc.vector.tensor_copy(
                        v_sb[:, kt, :].rearrange("p (h c) -> p h c", c=128)[:, :, 0:64],
                        psv.rearrange("p (h c) -> p h c", c=64),
                    )
                return emit

            def outproj_fillers(qb):
                """Two PE units: AllToAll gather + out-projection halves."""
                state = {}

                def f0():
                    af = afpool.tile([128, 8, 128], BF16, tag="af", name=f"af_{qb}")
                    for z in range(2):
                        nc.gpsimd.dma_start(
                            af[:, :, z * 64 : (z + 1) * 64],
                            a2a_out_d[qb][4 * z : 4 * z + 4]
                            .rearrange("s (r p) q -> p (s r) q", p=128),
                        )
                    osb_t = opool.tile([128, D], BF16, tag="osb", name=f"osb_{qb}")
                    state["af"], state["osb"] = af, osb_t
                    po = ps512.tile([128, 512], F32, tag="pp", name=f"po_{qb}_0")
                    for k in range(8):
                        nc.tensor.matmul(
                            po[:],
                            af[:, k, :],
                            wo_sb[:, k, 0:512],
                            start=(k == 0),
                            stop=(k == 7),
                        )
                    nc.vector.tensor_copy(osb_t[:, 0:512], po[:])

                def f1():
                    af, osb_t = state["af"], state["osb"]
                    po = ps512.tile([128, 512], F32, tag="pp", name=f"po_{qb}_1")
                    for k in range(8):
                        nc.tensor.matmul(
                            po[:],
                            af[:, k, :],
                            wo_sb[:, k, 512:D],
                            start=(k == 0),
                            stop=(k == 7),
                        )
                    nc.vector.tensor_copy(osb_t[:, 512:D], po[:])
                    nc.gpsimd.dma_start(out_d[qb * 128 : (qb + 1) * 128, :], osb_t[:])

                return [f0, f1]

            def attention_block(qb, fillers=(), chunk_cb=None, pvs_ref=None):
                """Causal attention for query block qb (keys 0..(qb+1)*512).

                Pops one pending filler unit after each kt iteration (evenly
                spread) so the PE queue interleaves ready full-array matmuls
                with the exp-gated attention matmuls.  chunk_cb(c) fires once
                128-query chunk c's last PV is emitted (diagonal pipelining).
                """
                fillers = list(fillers)
                popped = 0
                pvs = [
                    ppvp.tile([128, 512], F32, tag=f"pv{h}", name=f"pv{h}_{qb}")
                    for h in range(HPC)
                ]
                if pvs_ref is not None:
                    pvs_ref[0] = pvs
                nkt = 4 * qb + 4

                # PV runs one wave behind: the PE queue goes
                # [sc(w)][PV(w-1)][sc(w+1)][PV(w)]... so exp(w) (ACT) overlaps
                # PV(w-1) instead of serializing with it.
                pend_pv = []  # (kt, w, pt, qlo)

                def flush_pv():
                    while pend_pv:
                        pkt, pw, ppt, pqlo = pend_pv.pop(0)
                        for hh in range(2):
                            h = 2 * pw + hh
                            nc.tensor.matmul(
                                pvs[h][:, pqlo:512],
                                v_sb[:, pkt, 128 * h : 128 * h + 128],
                                ppt[:, hh, pqlo:512],
                                start=(pkt == 0),
                                stop=(pkt == nkt - 1),
                            )

                for kt in range(nkt):
                    j = kt - 4 * qb  # >= 0 on diagonal tiles
                    qlo = max(0, j * 128)
                    g0 = qb * 512 + qlo
                    g1 = (qb + 1) * 512
                    for w in range(2):
                        psc = pscp.tile(
                            [128, 2, 512], F32, tag="psc", name=f"psc{w}_{qb}_{kt}"
                        )
                        for hh in range(2):
                            nc.tensor.matmul(
                                psc[:, hh, qlo:512],
                                km[w][
                                    64 * hh : 64 * hh + 64,
                                    kt * 128 : (kt + 1) * 128,
                                ],
                                qm[w][64 * hh : 64 * hh + 64, g0:g1],
                                start=True,
                                stop=True,
                                tile_position=(64 * hh, 0),
                            )
                        pt = ptpool.tile(
                            [128, 2, 512], BF16, tag="pt", name=f"pt{w}_{qb}_{kt}"
                        )
                        nc.scalar.activation(
                            pt[:, :, qlo:512], psc[:, :, qlo:512], AF.Exp, scale=0.125
                        )
                        if j >= 0:
                            # causal mask on GpSimd: keeps the DVE queue free
                            # of ACT-dependent work (PV-drain ordering)
                            nc.gpsimd.tensor_tensor(
                                pt[:, :, qlo : qlo + 128],
                                pt[:, :, qlo : qlo + 128],
                                tri2_v,
                                ALU.mult,
                            )
                        flush_pv()
                        if chunk_cb is not None and j >= 1 and w == 0:
                            # chunk j-1's last PV was just flushed (w0)
                            chunk_cb(j - 1)
                        pend_pv.append((kt, w, pt, qlo))
                    while fillers and popped < (kt + 1) * len(fillers) // nkt:
                        fillers[popped]()
                        popped += 1
                flush_pv()
                if chunk_cb is not None:
                    chunk_cb(3)
                while popped < len(fillers):
                    fillers[popped]()
                    popped += 1
                return pvs

            def make_chunk_cb(qb, pvs_ref):
                """Per-128-query-chunk finalize: drain + recip + normalize +
                stage, pipelined with the block's remaining diagonal tiles.
                The AllToAll launches right after chunk 3."""

                def cb(c):
                    pvs = pvs_ref[0]
                    q0 = qb * 512 + c * 128
                    sl = slice(q0, q0 + 128)
                    for h in range(HPC):
                        nc.vector.tensor_copy(
                            attg_sb[:, h, sl], pvs[h][0:65, c * 128 : (c + 1) * 128]
                        )
                    sums_t = bcpool.tile(
                        [HPC, 128], BF16, tag="sums", name=f"sums_{qb}_{c}"
                    )
                    nc.sync.dma_start(sums_t[:], attg_sb[64:65, :, sl])
                    rb = bcpool.tile([HPC, 128], BF16, tag="rb", name=f"rb_{qb}_{c}")
                    with nc.allow_low_precision(reason="bf16 softmax denom recip"):
                        nc.vector.reciprocal(rb[:], sums_t[:])
                    rrow = bcpool.tile(
                        [1, HPC, 128], BF16, tag="rrow", name=f"rr_{qb}_{c}"
                    )
                    nc.sync.dma_start(rrow[:], rb[:])
                    bcx = bcpool.tile(
                        [64, HPC, 128], BF16, tag="bcx", name=f"bcx_{qb}_{c}"
                    )
                    nc.gpsimd.partition_broadcast(bcx[:], rrow[:])
                    for h in range(HPC):
                        slh = attg_sb[0:64, h, sl]
                        nc.vector.tensor_tensor(slh, slh, bcx[:, h, :], ALU.mult)
                    # stage the two AllToAll chunks covered by these queries
                    for p in (2 * c, 2 * c + 1):
                        nc.sync.dma_start(
                            a2a_in_d[qb][p].rearrange("(h l) q -> l h q", l=64),
                            attg_sb[
                                0:64, :, qb * 512 + p * 64 : qb * 512 + (p + 1) * 64
                            ],
                        )
                    if c == 3:
                        nc.gpsimd.collective_compute(
                            "AllToAll",
                            ALU.bypass,
                            replica_groups=A2A_GROUPS,
                            ins=[a2a_in_d[qb][:]],
                            outs=[a2a_out_d[qb][:]],
                        )

                return cb

            def wo_unit():
                nc.gpsimd.dma_start(
                    wo_sb[:], wo_d.ap().rearrange("(o p) m -> p o m", p=128)
                )

            # ---- emission ----
            # Prologue: block-0 Q/K projections + RoPE/merge + first V tile;
            # the ACT exp-table preload runs while the PE is dense.
            for idx in range(4):
                qk_unit(0, idx)()
            warm_act = cpool.tile([128, 1], F32, tag="warm_act")
            nc.scalar.activation(warm_act[:], bq0_sb[:], AF.Exp)
            rope_merge_unit(0)()
            load_x_rest()
            v_unit(0)()

            # attention(st) carries the remaining V tiles, the next block's
            # projection units, and trailing out-projections as fillers; the
            # per-chunk callback drains/normalizes/stages as the causal
            # diagonal completes and fires the AllToAll at chunk 3.
            ref = [None]
            attention_block(
                0,
                [v_unit(kt) for kt in range(1, 4)]
                + [qk_unit(1, i) for i in range(4)]
                + [rope_merge_unit(1)]
                + [v_unit(kt) for kt in range(4, 8)],
                chunk_cb=make_chunk_cb(0, ref),
                pvs_ref=ref,
            )
            ref = [None]
            attention_block(
                1,
                [qk_unit(2, i) for i in range(4)]
                + [rope_merge_unit(2), wo_unit]
                + [v_unit(kt) for kt in range(8, 12)],
                chunk_cb=make_chunk_cb(1, ref),
                pvs_ref=ref,
            )
            ref = [None]
            attention_block(
                2,
                [qk_unit(3, i) for i in range(4)]
                + [rope_merge_unit(3)]
                + [v_unit(kt) for kt in range(12, 16)]
                + outproj_fillers(0),
                chunk_cb=make_chunk_cb(2, ref),
                pvs_ref=ref,
            )
            ref = [None]
            attention_block(
                3,
                outproj_fillers(1) + outproj_fillers(2),
                chunk_cb=make_chunk_cb(3, ref),
                pvs_ref=ref,
            )
            for f in outproj_fillers(3):
                f()

    nc.compile()
    return nc


def make_in_maps(x, Wq, bq, Wk, bk, Wv, bv, Wo):
    inv = 1.0 / (ROPE_BASE ** (2.0 * np.arange(32, dtype=np.float64) / L))
    ang = np.arange(S, dtype=np.float64)[:, None] * inv[None, :]  # [S, 32]
    cos4 = np.tile(np.cos(ang).T, (HPC, 1)).astype(NPBF16)  # [128, S]
    sin4 = np.tile(np.sin(ang).T, (HPC, 1)).astype(NPBF16)
    tri = (np.arange(128)[None, :] >= np.arange(128)[:, None]).astype(NPBF16)
    tri2 = np.repeat(tri[:, None, :], 2, axis=1).reshape(128, 256)
    wo_full = np.ascontiguousarray(Wo).astype(NPBF16)

    in_maps = []
    for c in range(N_CORES):
        b, g = divmod(c, HPC)
        even = np.concatenate(
            [64 * h + 2 * np.arange(32) for h in range(4 * g, 4 * g + 4)]
        )
        odd = even + 1
        vcols = np.arange(256 * g, 256 * (g + 1))
        in_maps.append(
            {
                "xt": np.ascontiguousarray(x[b].T).astype(NPBF16),
                "wq0": np.ascontiguousarray(Wq[:, even]).astype(NPBF16),
                "wq1": np.ascontiguousarray(Wq[:, odd]).astype(NPBF16),
                "wk0": np.ascontiguousarray(Wk[:, even]).astype(NPBF16),
                "wk1": np.ascontiguousarray(Wk[:, odd]).astype(NPBF16),
                "wv": np.ascontiguousarray(Wv[:, vcols]).astype(NPBF16),
                "wo": wo_full,
                "bq0": bq[even].reshape(128, 1).astype(np.float32),
                "bq1": bq[odd].reshape(128, 1).astype(np.float32),
                "bk0": bk[even].reshape(128, 1).astype(np.float32),
                "bk1": bk[odd].reshape(128, 1).astype(np.float32),
                "bvr": bv[vcols].reshape(1, 256).astype(NPBF16),
                "cos4": cos4,
                "sin4": sin4,
                "tri2": tri2,
            }
        )
    return in_maps


def assemble_output(results, bo):
    out = np.empty((B, S, D), np.float32)
    for c in range(N_CORES):
        # core c owns rows qb*512 + c*64 .. +64 of BOTH batches
        sh = np.asarray(results[c]["out"]).astype(np.float32).reshape(NQB, 2, 64, D)
        for qb in range(NQB):
            r0 = qb * 512 + c * 64
            out[0, r0 : r0 + 64, :] = sh[qb, 0]
            out[1, r0 : r0 + 64, :] = sh[qb, 1]
    # bo is added once, after the reduction (matches `attended @ Wo + bo`).
    out += bo[None, None, :].astype(np.float32)
    return out


_CACHE = {}


def kernel(x, Wq, bq, Wk, bk, Wv, bv, Wo, bo, **run_kwargs):
    if "nc" not in _CACHE:
        _CACHE["nc"] = build_program()
    nc = _CACHE["nc"]
    in_maps = make_in_maps(
        np.asarray(x), np.asarray(Wq), np.asarray(bq), np.asarray(Wk),
        np.asarray(bk), np.asarray(Wv), np.asarray(bv), np.asarray(Wo),
    )
    res = bass_utils.run_bass_kernel_spmd(
        nc, in_maps, core_ids=list(range(N_CORES)), **run_kwargs
    )
    out = assemble_output(res.results, np.asarray(bo))
    kernel.last_results = res
    return out
